# revision 9
# baseline (speedup 1.0000x reference)
"""9x9 morphological dilation (sliding-window max, SAME padding) on Trainium2.

Input : label (16, 1024, 1024, 1) float32, values in [0, 1).
Output: same shape; out[b,i,j] = max over the 9x9 window centered at (i,j),
        clipped to the image (cv2-style border handling for dilate).

v5 strategy (per NeuronCore; batch data-parallel over 8 cores, 2 images/core):
  - All device compute and I/O in bf16.  max() is monotone, so casting the
    input to bf16 on the host and taking exact bf16 maxes gives the bf16
    rounding of the true max: rel err <= 2^-9 ~ 0.2%, far inside the 2e-2
    gate.  Halves DMA traffic and doubles DVE throughput (2x_1p mode).
  - SBUF tile layout: 128 partitions x (24 rows x U cols), U = cw + 8.
    Partition p holds img = p//64, row-block q = p%64, loaded with image
    rows [16q-4, 16q+19] via an overlapping DRAM AP.  The symmetric
    +-4-row halo makes every partition self-contained: R9[r] = vertical
    9-max centered on output row 16q+r.  Image borders are zero padding
    (valid stand-in for -inf since inputs >= 0).
  - X tiles are persistent (one per chunk, allocated outside the loop) and
    fully zeroed once at startup; the pad regions are never overwritten by
    loads, so no per-chunk memsets or carry DMAs exist at steady state.
    (HWDGE descriptor-gen is a serial shared resource at ~630ns per DMA
    instruction, and DVE-class ops cannot even start at partition 63, so
    pad handling via persistent zeros is both fastest and simplest.)
  - Vertical 9-max: log tree, shifts +1,+2,+4,+1 rows (bf16 2x tensor_max).
  - Horizontal 9-max: log tree, shifts +1,+2,+4 cols then +8 vs R9.
    Tile col j holds image col c0+j-4; OUT[j] = max over image cols
    [c0+j-4, c0+j+4] = output col c0+j, j in [0, cw).
  - Stores are one [64,16,cw] DMA per image per chunk.
  - Emission is software-pipelined: chunk i's horizontal stage is emitted
    after chunk i+1's vertical tree; loads prefetch one chunk ahead on the
    ACT HWDGE ring; stores ride the SP ring.
"""

import numpy as np

B, H, W = 16, 1024, 1024
NCORES = 8
IMGS = 2            # images per core
RB = 16             # output rows per partition
RBH = RB + 8        # loaded rows per partition (+-4 halo)
CHUNKS = [512, 512]  # output cols per chunk (sum = 1024)
assert sum(CHUNKS) == W

_CACHE = {}


def _build(reps=1):
    import concourse.bacc as bacc
    import concourse.tile as tile
    import concourse.mybir as mybir
    from concourse.ap import AP

    bf16 = mybir.dt.bfloat16

    nc = bacc.Bacc("TRN2", target_bir_lowering=False, debug=False, num_devices=1)
    xt = nc.dram_tensor("x", [IMGS, H, W], bf16, kind="ExternalInput")
    y = nc.dram_tensor("y", [IMGS, H, W], bf16, kind="ExternalOutput").ap()
    # distinct shape per reps so the PJRT/neuron compile caches can never
    # collide across reps variants (their keying ignores backend_config)
    nc.dram_tensor("rtag", [1, 8 * reps], bf16, kind="ExternalInput")

    chunk_off = np.cumsum([0] + CHUNKS[:-1]).tolist()
    nch = len(CHUNKS)

    with tile.TileContext(nc) as tc:
        with (
            tc.tile_pool(name="px", bufs=1) as px,
            tc.tile_pool(name="pa", bufs=2) as pa,
            tc.tile_pool(name="pb", bufs=3) as pb,
            tc.tile_pool(name="pd", bufs=2) as pd,
        ):
            # persistent per-chunk input tiles, fully zeroed once: the zero
            # pad rows/cols are never overwritten by the loads below
            xtiles = []
            for ch in range(nch):
                u = CHUNKS[ch] + 8
                X = px.tile([128, RBH * u], bf16, tag=f"x{ch}", name=f"X{ch}")
                nc.vector.memset(X[:], 0.0)
                xtiles.append(X.rearrange("p (r u) -> p r u", u=u))

            def emit_load(ch, first=False):
                cw = CHUNKS[ch]
                u = cw + 8
                c0 = chunk_off[ch]
                clo = max(0, c0 - 4)
                chi = min(W, c0 + cw + 4)
                ncols = chi - clo
                ulo = clo - (c0 - 4)
                x3 = xtiles[ch]
                halves = [(0, 12), (12, RBH)] if first else [(0, RBH)]
                for img in range(IMGS):
                    b = 64 * img
                    base = img * H * W + clo
                    for rlo, rhi in halves:
                        # q 1..62: full 24 overlapping rows (16q-4 .. 16q+19)
                        nc.scalar.dma_start(
                            out=x3[b + 1:b + 63, rlo:rhi, ulo:ulo + ncols],
                            in_=AP(xt, base + (16 - 4 + rlo) * W,
                                   [[RB * W, 62], [W, rhi - rlo], [1, ncols]]),
                        )
                    # q == 0: image rows 0..19 into tile rows 4..23
                    nc.scalar.dma_start(
                        out=x3[b:b + 1, 4:RBH, ulo:ulo + ncols],
                        in_=AP(xt, base, [[RB * W, 1], [W, RB + 4], [1, ncols]]),
                    )
                    # q == 63: image rows 1004..1023 into tile rows 0..19
                    nc.scalar.dma_start(
                        out=x3[b + 63:b + 64, 0:RB + 4, ulo:ulo + ncols],
                        in_=AP(xt, base + (63 * RB - 4) * W,
                               [[RB * W, 1], [W, RB + 4], [1, ncols]]),
                    )

            def emit_tree(it):
                ch = it % nch
                cw = CHUNKS[ch]
                u = cw + 8
                x3 = xtiles[ch]

                T2 = pa.tile([128, (RBH - 1) * u], bf16, tag="a", name="T2")
                t2_3 = T2.rearrange("p (r u) -> p r u", u=u)
                if it == 0:
                    # first chunk: start on the first loaded half while the
                    # rest of the very first load is still in flight
                    nc.vector.tensor_max(t2_3[:, 0:11, :], x3[:, 0:11, :], x3[:, 1:12, :])
                    nc.vector.tensor_max(t2_3[:, 11:23, :], x3[:, 11:23, :], x3[:, 12:24, :])
                else:
                    nc.vector.tensor_max(t2_3[:, 0:23, :], x3[:, 0:23, :], x3[:, 1:24, :])

                T4 = pb.tile([128, (RBH - 3) * u], bf16, tag="b", name="T4")
                t4_3 = T4.rearrange("p (r u) -> p r u", u=u)
                nc.vector.tensor_max(t4_3[:, 0:21, :], t2_3[:, 0:21, :], t2_3[:, 2:23, :])

                T8 = pa.tile([128, (RBH - 7) * u], bf16, tag="a", name="T8")
                t8_3 = T8.rearrange("p (r u) -> p r u", u=u)
                nc.vector.tensor_max(t8_3[:, 0:17, :], t4_3[:, 0:17, :], t4_3[:, 4:21, :])

                R9 = pb.tile([128, RB * u], bf16, tag="b", name="R9")
                r9_3 = R9.rearrange("p (r u) -> p r u", u=u)
                nc.vector.tensor_max(r9_3[:, 0:16, :], t8_3[:, 0:16, :], t8_3[:, 1:17, :])
                return (R9, r9_3)

            def emit_hstage(it, R9, r9_3, last=False):
                ch = it % nch
                cw = CHUNKS[ch]
                u = cw + 8
                fs = RB * u
                c0 = chunk_off[ch]
                # horizontal log tree over columns
                H2 = pa.tile([128, fs], bf16, tag="a", name="H2")
                h2 = H2.rearrange("p (r u) -> p r u", u=u)
                nc.vector.tensor_max(h2[:, :, 0:cw + 7], r9_3[:, :, 0:cw + 7], r9_3[:, :, 1:cw + 8])
                H4 = pd.tile([128, fs], bf16, tag="d", name="H4")
                h4 = H4.rearrange("p (r u) -> p r u", u=u)
                nc.vector.tensor_max(h4[:, :, 0:cw + 4], h2[:, :, 0:cw + 4], h2[:, :, 2:cw + 6])

                OUT = pd.tile([128, fs], bf16, tag="d", name="OUT")
                o3 = OUT.rearrange("p (r u) -> p r u", u=u)
                H8 = pa.tile([128, fs], bf16, tag="a", name="H8")
                h8 = H8.rearrange("p (r u) -> p r u", u=u)
                st = nc.sync
                groups = [(0, 8), (8, 16)] if last else [(0, 16)]
                for r0g, r1g in groups:
                    nc.vector.tensor_max(
                        h8[:, r0g:r1g, 0:cw], h4[:, r0g:r1g, 0:cw], h4[:, r0g:r1g, 4:cw + 4]
                    )
                    nc.vector.tensor_max(
                        o3[:, r0g:r1g, 0:cw], h8[:, r0g:r1g, 0:cw], r9_3[:, r0g:r1g, 8:cw + 8]
                    )
                    for img in range(IMGS):
                        b = 64 * img
                        ymain = y[img][:, c0:c0 + cw].rearrange("(q r) c -> q r c", r=RB)
                        st.dma_start(
                            out=ymain[:, r0g:r1g, :], in_=o3[b:b + 64, r0g:r1g, 0:cw]
                        )

            niter = nch * reps
            emit_load(0, first=True)
            pending = None
            for it in range(niter):
                if it + 1 < niter:
                    emit_load((it + 1) % nch)
                state = emit_tree(it)
                if pending is not None:
                    emit_hstage(*pending)
                pending = (it, *state)
            emit_hstage(*pending, last=True)

    nc.compile()
    return nc


def kernel(label):
    import ml_dtypes

    lab = np.asarray(label, dtype=np.float32).reshape(B, H, W)
    lab16 = np.ascontiguousarray(lab.astype(ml_dtypes.bfloat16))
    if "nc" not in _CACHE:
        _CACHE["nc"] = _build()
    nc = _CACHE["nc"]

    from concourse.bass_utils import run_bass_kernel_spmd

    rtag = np.zeros((1, 8), ml_dtypes.bfloat16)
    in_maps = [
        {"x": lab16[IMGS * c:IMGS * (c + 1)], "rtag": rtag} for c in range(NCORES)
    ]
    res = run_bass_kernel_spmd(nc, in_maps, core_ids=list(range(NCORES)))
    out = np.concatenate([res.results[c]["y"] for c in range(NCORES)], axis=0)
    return out.astype(np.float32).reshape(B, H, W, 1)
